# revision 38
# baseline (speedup 1.0000x reference)
"""Additive (Bahdanau) attention scores on 8 Trainium2 NeuronCores.

scores[b,h,q,k] = sum_d V[d]*tanh((Q@W1+b1)[b,h,q,d] + (K@W2+b2)[b,h,k,d]) + bV

Strategy: tanh(x) on x in [-6,6] is approximated by a 7-term sine sum
    tanh(x) ~= sum_j AL[j]*sin(OM[j]*x)        (minimax err ~5.2e-5)
and sin(w*(a+b)) separates: sin(wa+p1)cos(wb+p2) + cos(wa+p1)sin(wb+p2)
with p1+p2 = 0 mod 2pi.  So with atoms
    A_j[(rep,d), q] = [sin(w_j a_qd + w_j b1_d); cos(...)]        (rep 0/1)
    B_j[(rep,d), k] = AL_j V_d [cos(w_j b_kd + w_j b2_d); sin(...)]
we get scores = sum_j A_j^T B_j + bV: 7 accumulating 128-contraction
matmuls per 128x512 output tile on the PE (fp16 operands, fp32 psum).

The scalar engine's Sin only accepts [-pi, pi], so atoms with larger
phase range are range-reduced first with one fused tensor_scalar:
    m = (u + c'_j) mod (2pi/w_j)   then   atom = Sin(w_j*m - pi)
which equals sin(w_j*u + c_j) exactly (c'_j=(c_j+pi+2pi*M_j)/w_j keeps
the mod argument positive).

Sharding: data-parallel over the 16 (b,h) pairs, 2 per core.
"""

import sys

for _p in ("/opt/trn_rl_repo",):
    if _p not in sys.path:
        sys.path.insert(0, _p)

import numpy as np

import concourse.bass as bass
import concourse.tile as tile
from concourse.tile import add_dep_helper
from concourse import mybir
from concourse.bass_utils import run_bass_kernel_spmd
_MAGIC = 12582912.0  # 1.5 * 2**23: fp32 add/sub rounds to nearest integer

# Free-frequency sine fits of tanh on [-R, R]: R -> (omegas, alphas).
# Generated offline (minimax ~5.2e-5 for R=6).
FITS = {
    6.0: (
        np.array([0.39470029585086247, 1.2008812193188088, 2.045755849154067,
                  2.934844618886767, 3.867983838961661, 4.843375387328973]),
        np.array([1.1998422653874294, 0.2556250274358582, 0.06983809829990795,
                  0.01813741512739277, 0.004388615240772713,
                  0.0010311960518593858]),
    ),
}
# J=7 alternative (minimax 5.2e-5): om=[0.38945552149763957, 1.184209496208925,
# 2.0154944303508757, 2.888278870385511, 3.8019292386446444, 4.756096897781546,
# 5.75056160605657], al=[1.2016194017620094, 0.25878757556077875,
# 0.0719935566397645, 0.019132206116179717, 0.004760899364273443,
# 0.001109882782129597, 0.0002521535044474215]

N_CORES = 8
HPC = 2          # (b*h) heads per core: 16 / 8
LQ = 512
LK = 512
D = 64
QT = LQ // 128   # q tiles per head
TWO_PI = 2.0 * np.pi
MARGIN = 0.02    # stay this far inside [-pi, pi] for direct (no-mod) atoms


def _plan(b1, b2, u_bound_a, u_bound_b, R_need):
    """Compute per-atom constants. Returns (om, al, J, cc[128,2J], plan[2J])
    where plan[col] = ("direct", omega) or ("mod", omega, P)."""
    Rs = sorted(FITS.keys())
    R_fit = None
    for r in Rs:
        if r >= R_need:
            R_fit = r
            break
    if R_fit is None:
        R_fit = Rs[-1]
    om, al = FITS[R_fit]
    J = len(om)

    b1d = np.concatenate([b1, b1]).astype(np.float64)
    b2d = np.concatenate([b2, b2]).astype(np.float64)
    phaseA = np.concatenate([np.zeros(64), np.full(64, np.pi / 2)])
    phaseB = np.concatenate([np.full(64, np.pi / 2), np.zeros(64)])

    cc = np.empty((128, 2 * J), np.float32)
    plan = []
    for col in range(2 * J):
        j = col % J
        w = float(om[j])
        if col < J:
            c = w * b1d + phaseA
            ub = u_bound_a
        else:
            c = w * b2d + phaseB
            ub = u_bound_b
        if w * ub + np.abs(c).max() <= np.pi - MARGIN:
            cc[:, col] = c.astype(np.float32)
            plan.append(("direct", w))
        else:
            cc[:, col] = ((c / TWO_PI + 0.5) * 262144.0).astype(np.float32)
            plan.append(("fold", w))
    return om, al, J, cc, plan


NBLK = HPC * QT + 3  # 8 transposed data tiles + [W1dup|W2dup, cc, vcoef]
BLK_W = HPC * QT
BLK_CC = HPC * QT + 1
BLK_VC = HPC * QT + 2


def build_nc(bV_val, J, plan):
    f32 = mybir.dt.float32
    f16 = mybir.dt.float16
    SIN = mybir.ActivationFunctionType.Sin

    nc = bass.Bass()
    # qk: [128, NBLK, 128] f32. Blocks 0..7: [p,(h,t),0:64]=Q tile,
    # [...,64:128]=K tile. Then identity / W1dup / W2dup / cc / vcoef
    # blocks, so the whole constant+input set arrives in ONE DMA (single
    # semaphore -> every PE instruction needs at most one wait per operand).
    qk = nc.declare_dram_parameter("qk", [128, NBLK, 128], f32, isOutput=False)
    # out[h, p, qc, k] = scores[h, qc*128+p, k]
    out = nc.declare_dram_parameter("out", [HPC, 128, QT, LK], f32, isOutput=True)

    with tile.TileContext(nc) as tc:
        spsum_cm = tc.tile_pool(name="spsum", bufs=2, space="PSUM")
        spsum = spsum_cm.__enter__()
        ppsum_cm = tc.tile_pool(name="ppsum", bufs=1, space="PSUM")
        ppsum = ppsum_cm.__enter__()
        with (
            tc.tile_pool(name="inp", bufs=1) as inp,
            tc.tile_pool(name="qkt", bufs=1) as qkt_pool,
            tc.tile_pool(name="proj", bufs=1) as proj_pool,
            tc.tile_pool(name="marg", bufs=max(1, sum(1 for p in plan if p[0] != "direct"))) as marg_pool,
            tc.tile_pool(name="mm", bufs=max(1, sum(1 for p in plan if p[0] != "direct"))) as mm_pool,
            tc.tile_pool(name="sout2", bufs=1) as sout2_pool,
            tc.tile_pool(name="atoms", bufs=J) as atom_pool,
            tc.tile_pool(name="braw", bufs=J) as braw_pool,
            tc.tile_pool(name="sout", bufs=1) as sout_pool,
        ):
            insts = {"PE": [], "ACT": [], "DVE": [], "POOL": [], "DMA": []}
            qk_sb = inp.tile([128, NBLK, 128], f32)
            insts["DMA"].append(nc.sync.dma_start(out=qk_sb, in_=qk[:, :, :]))

            # Warm-up touches: one tiny instruction per engine that reads
            # qk_sb, so each engine observes the input-DMA semaphore early
            # and later instructions carry at most ONE new wait (several
            # instruction structs have a single sync-wait slot).
            warm = inp.tile([128, 3], f32, tag="warm")
            insts["POOL"].append(
                nc.gpsimd.tensor_copy(warm[:, 0:1], qk_sb[:, BLK_CC, 0:1]))
            insts["DVE"].append(
                nc.vector.tensor_copy(warm[:, 1:2], qk_sb[:, BLK_CC, 0:1]))
            insts["ACT"].append(
                nc.scalar.copy(warm[:, 2:3], qk_sb[:, BLK_CC, 0:1]))

            # Q^T lives in partitions 0-63 of the data blocks, K^T in
            # 64-127 (host pre-transposed).  Projections contract straight
            # out of the input tile; W2dup sits in partitions 64-127 of the
            # weight block so lhsT/rhs partition bases match.
            aT2 = ppsum.tile([128, HPC, LQ], f32, tag="aT2")
            bT2 = ppsum.tile([128, HPC, LK], f32, tag="bT2")
            for h in range(HPC):
                insts["PE"].append(nc.tensor.matmul(
                    aT2[:, h, :], lhsT=qk_sb[0:64, BLK_W, :],
                    rhs=qk_sb[0:64, h * QT:(h + 1) * QT, :],
                    start=True, stop=True))
                insts["PE"].append(nc.tensor.matmul(
                    bT2[:, h, :], lhsT=qk_sb[64:128, BLK_W, :],
                    rhs=qk_sb[64:128, h * QT:(h + 1) * QT, :],
                    start=True, stop=True))
            aT2_sb = proj_pool.tile([128, HPC * LQ], f32, tag="aT2_sb")
            insts["DVE"].append(nc.vector.tensor_copy(aT2_sb, aT2))
            bT2_sb = proj_pool.tile([128, HPC * LK], f32, tag="bT2_sb")
            insts["DVE"].append(nc.vector.tensor_copy(bT2_sb, bT2))
            ppsum_cm.__exit__(None, None, None)
            spsum2_cm = tc.tile_pool(name="spsum2", bufs=6, space="PSUM")
            spsum2 = spsum2_cm.__enter__()

            negpi = qk_sb[:, BLK_CC, 2 * J:2 * J + 1]
            bvcol = qk_sb[:, BLK_CC, 2 * J + 1:2 * J + 2]
            i32 = mybir.dt.int32
            FSC = 262144.0  # 2^18 phase quantization
            fold_ctr = [0]

            def make_atom(dst, src_sb, col, mod_engine):
                kind = plan[col][0]
                w = plan[col][1]
                cvec = qk_sb[:, BLK_CC, col:col + 1]
                if kind == "direct":
                    insts["ACT"].append(
                        nc.scalar.activation(dst, src_sb, SIN,
                                             bias=cvec, scale=float(w)))
                else:
                    # w32 = i32(u*(2^18*w/2pi) + (c/2pi + .5)*2^18)  [Pool/DVE]
                    # m32 = w32 & 0x3FFFF                            [DVE]
                    # atom = sin((2pi/2^18)*m32 - pi)                [ACT]
                    y = marg_pool.tile([128, HPC * LQ], i32, tag="my")
                    fold_ctr[0] += 1
                    eng, key = ((nc.gpsimd, "POOL") if fold_ctr[0] % 3
                                else (nc.vector, "DVE"))
                    insts[key].append(eng.tensor_scalar(
                        out=y, in0=src_sb,
                        scalar1=float(FSC * w / TWO_PI), scalar2=cvec,
                        op0=mybir.AluOpType.mult, op1=mybir.AluOpType.add))
                    m = mm_pool.tile([128, HPC * LQ], i32, tag="mm")
                    insts["DVE"].append(nc.vector.tensor_scalar(
                        out=m, in0=y, scalar1=0x3FFFF, scalar2=None,
                        op0=mybir.AluOpType.bitwise_and))
                    insts["ACT"].append(
                        nc.scalar.activation(dst, m, SIN,
                                             bias=negpi, scale=float(TWO_PI / FSC)))

            atomsA = []
            atomsB = []
            for j in range(J):
                aA = atom_pool.tile([128, HPC * LQ], f16, tag="atomA")
                make_atom(aA, aT2_sb, j, nc.vector)
                bR = braw_pool.tile([128, HPC * LK], f16, tag="braw")
                make_atom(bR, bT2_sb, J + j, nc.vector)
                aB = atom_pool.tile([128, HPC * LK], f16, tag="atomB")
                insts["DVE"].append(nc.vector.tensor_scalar_mul(
                    aB, bR, qk_sb[:, BLK_VC, j:j + 1]))
                atomsA.append(aA)
                atomsB.append(aB)

            IDENT = mybir.ActivationFunctionType.Identity
            so_h0 = sout_pool.tile([128, QT, LK], f32, tag="so")
            so_h1 = sout2_pool.tile([128, QT, LK], f32, tag="so2")
            sos = [so_h0, so_h1]
            gi = 0
            for h in range(HPC):
                so = sos[h]
                for qc in range(QT):
                    # groups 0-1 and 6-7 use the always-reserved banks;
                    # groups 2-5 take the banks released by tpsum/ppsum
                    # (first reuse happens far from those banks' writers,
                    # so the implied waits collapse to one).
                    if gi < 2:
                        sps = spsum.tile([128, LK], f32, tag="sps")
                    else:
                        sps = spsum2.tile([128, LK], f32, tag="sps2")
                    gi += 1
                    for j in range(J):
                        insts["PE"].append(nc.tensor.matmul(
                            sps,
                            lhsT=atomsA[j][:, h * LQ + qc * 128:
                                            h * LQ + (qc + 1) * 128],
                            rhs=atomsB[j][:, h * LK:(h + 1) * LK],
                            start=(j == 0), stop=(j == J - 1)))
                    if h == 0:
                        insts["DVE"].append(nc.vector.tensor_scalar_add(
                            so[:, qc, :], sps, float(bV_val)))
                    else:
                        insts["ACT"].append(nc.scalar.activation(
                            so[:, qc, :], sps, IDENT, bias=bvcol, scale=1.0))
                insts["DMA"].append(nc.sync.dma_start(out=out[h], in_=so))

            spsum2_cm.__exit__(None, None, None)
            spsum_cm.__exit__(None, None, None)
            # Collector nops: one per producer class, each absorbing one
            # semaphore into the sync engine's observed clock so the
            # framework tail drain needs no multi-sem wait (hardware allows
            # one sync-wait per instruction).
            for key in ("POOL", "ACT", "PE", "DVE"):
                if not insts[key]:
                    continue
                nop = nc.sync.nop(nofuse=True, hint=f"collect_{key}")
                for prod in insts[key]:
                    add_dep_helper(nop.ins, prod.ins, sync=True,
                                   reason=f"tail collector {key}")
            for i, prod in enumerate(insts["DMA"]):
                nop = nc.sync.nop(nofuse=True, hint=f"collect_dma{i}")
                add_dep_helper(nop.ins, prod.ins, sync=True,
                               reason="tail collector dma")
    return nc


def _prep_inputs(Q, K, W1, b1, W2, b2, V, bV):
    B, H, Lq, D_ = Q.shape
    BH = B * H
    Qf = np.ascontiguousarray(Q.reshape(BH, Lq, D_).astype(np.float32))
    Kf = np.ascontiguousarray(K.reshape(BH, Lq, D_).astype(np.float32))

    # data bounds for range-reduction planning (raw projections, bias excluded)
    a_raw = Qf.reshape(-1, D_) @ W1
    b_raw = Kf.reshape(-1, D_) @ W2
    ub_a = float(np.abs(a_raw).max()) + 0.05
    ub_b = float(np.abs(b_raw).max()) + 0.05
    R_need = (ub_a + np.abs(b1).max()) + (ub_b + np.abs(b2).max())

    om, al, J, cc, plan = _plan(b1, b2, ub_a, ub_b, R_need)

    consts = np.zeros((128, 3, 128), np.float32)
    consts[0:64, 0, :] = np.concatenate([W1, W1], axis=1)
    consts[64:128, 0, :] = np.concatenate([W2, W2], axis=1)
    consts[:, 1, 0:2 * J] = cc
    consts[:, 1, 2 * J] = -np.pi
    consts[:, 1, 2 * J + 1] = np.float32(bV[0])
    Vd = np.concatenate([V[:, 0], V[:, 0]])
    consts[:, 2, 0:J] = al[None, :] * Vd[:, None]

    in_maps = []
    for c in range(N_CORES):
        qk = np.empty((128, NBLK, 128), np.float32)
        for i in range(HPC):
            h = HPC * c + i
            qt = Qf[h].T
            kt = Kf[h].T
            for t in range(QT):
                qk[0:64, i * QT + t, :] = qt[:, t * 128:(t + 1) * 128]
                qk[64:128, i * QT + t, :] = kt[:, t * 128:(t + 1) * 128]
        qk[:, HPC * QT:, :] = consts
        in_maps.append({"qk": qk})
    return in_maps, J, plan


def _run(inputs, trace=False, **kwargs):
    Q = np.asarray(inputs["Q"], np.float32)
    K = np.asarray(inputs["K"], np.float32)
    W1 = np.asarray(inputs["W1"], np.float32)
    b1 = np.asarray(inputs["b1"], np.float32)
    W2 = np.asarray(inputs["W2"], np.float32)
    b2 = np.asarray(inputs["b2"], np.float32)
    V = np.asarray(inputs["V"], np.float32)
    bV = np.asarray(inputs["bV"], np.float32)

    in_maps, J, plan = _prep_inputs(Q, K, W1, b1, W2, b2, V, bV)
    nc = build_nc(float(bV[0]), J, plan)
    res = run_bass_kernel_spmd(nc, in_maps, list(range(N_CORES)),
                               trace=trace, **kwargs)

    B, H, Lq, _ = Q.shape
    out = np.empty((B * H, Lq, LK), np.float32)
    for c in range(N_CORES):
        o = res.results[c]["out"]          # [HPC, 128, QT, LK]
        out[HPC * c:HPC * (c + 1)] = (
            o.transpose(0, 2, 1, 3).reshape(HPC, Lq, LK))
    return out.reshape(B, H, Lq, LK), res


def kernel(**inputs) -> np.ndarray:
    out, _ = _run(inputs, trace=False)
    return out


# revision 39
# speedup vs baseline: 1.0052x; 1.0052x over previous
"""Additive (Bahdanau) attention scores on 8 Trainium2 NeuronCores.

scores[b,h,q,k] = sum_d V[d]*tanh((Q@W1+b1)[b,h,q,d] + (K@W2+b2)[b,h,k,d]) + bV

Strategy: tanh(x) on x in [-6,6] is approximated by a J-term sine sum
    tanh(x) ~= sum_j AL[j]*sin(OM[j]*x)        (minimax ~2.2e-4 at J=6)
and sin(w*(a+b)) separates: sin(wa+p1)cos(wb+p2) + cos(wa+p1)sin(wb+p2)
with p1+p2 = 0.  With fp16 atoms (rep 0/1 in partition halves)
    A_j[(rep,d), q] = [sin(w_j a_qd + w_j b1_d); cos(...)]
    B_j[(rep,d), k] = AL_j V_d [cos(w_j b_kd + w_j b2_d); sin(...)]
scores = sum_j A_j^T B_j + bV: J accumulating 128-contraction matmuls
per 128x512 output tile on the PE (fp32 psum).  End-to-end relative
error ~3e-4 (fp16 factor quantization dominates).

The scalar engine's Sin only accepts [-pi, pi]; atoms whose phase can
leave that range are range-reduced in integer turns:
    w32 = int32(u*(2^18*w/2pi) + (c/2pi + 0.5)*2^18)    [GpSimd/DVE]
    m32 = w32 & 0x3FFFF                                 [DVE]
    atom = Sin((2pi/2^18)*m32 - pi)                     [ACT, int32 in]
which equals sin(w*u + c) exactly up to 2.4e-5 rad quantization.

Q^T/K^T are pre-transposed on the host into the input blocks
(partitions 0-63 = Q^T, 64-127 = K^T), so the device does projections
directly.  Single input DMA carries data + all constants (one
semaphore); per-engine collector nops keep every instruction within
the hardware's single-sync-wait budget.

Sharding: data-parallel over the 16 (b,h) pairs, 2 per core.
"""

import sys

for _p in ("/opt/trn_rl_repo",):
    if _p not in sys.path:
        sys.path.insert(0, _p)

import numpy as np

import concourse.bass as bass
import concourse.tile as tile
from concourse.tile import add_dep_helper
from concourse import mybir
from concourse.bass_utils import run_bass_kernel_spmd
_MAGIC = 12582912.0  # 1.5 * 2**23: fp32 add/sub rounds to nearest integer

# Free-frequency sine fits of tanh on [-R, R]: R -> (omegas, alphas).
# Generated offline (minimax ~5.2e-5 for R=6).
FITS = {
    6.0: (
        np.array([0.39470029585086247, 1.2008812193188088, 2.045755849154067,
                  2.934844618886767, 3.867983838961661, 4.843375387328973]),
        np.array([1.1998422653874294, 0.2556250274358582, 0.06983809829990795,
                  0.01813741512739277, 0.004388615240772713,
                  0.0010311960518593858]),
    ),
}
# J=7 alternative (minimax 5.2e-5): om=[0.38945552149763957, 1.184209496208925,
# 2.0154944303508757, 2.888278870385511, 3.8019292386446444, 4.756096897781546,
# 5.75056160605657], al=[1.2016194017620094, 0.25878757556077875,
# 0.0719935566397645, 0.019132206116179717, 0.004760899364273443,
# 0.001109882782129597, 0.0002521535044474215]

N_CORES = 8
HPC = 2          # (b*h) heads per core: 16 / 8
LQ = 512
LK = 512
D = 64
QT = LQ // 128   # q tiles per head
TWO_PI = 2.0 * np.pi
MARGIN = 0.02    # stay this far inside [-pi, pi] for direct (no-mod) atoms


def _plan(b1, b2, u_bound_a, u_bound_b, R_need):
    """Compute per-atom constants. Returns (om, al, J, cc[128,2J], plan[2J])
    where plan[col] = ("direct", omega) or ("mod", omega, P)."""
    Rs = sorted(FITS.keys())
    R_fit = None
    for r in Rs:
        if r >= R_need:
            R_fit = r
            break
    if R_fit is None:
        R_fit = Rs[-1]
    om, al = FITS[R_fit]
    J = len(om)

    b1d = np.concatenate([b1, b1]).astype(np.float64)
    b2d = np.concatenate([b2, b2]).astype(np.float64)
    phaseA = np.concatenate([np.zeros(64), np.full(64, np.pi / 2)])
    phaseB = np.concatenate([np.full(64, np.pi / 2), np.zeros(64)])

    cc = np.empty((128, 2 * J), np.float32)
    plan = []
    for col in range(2 * J):
        j = col % J
        w = float(om[j])
        if col < J:
            c = w * b1d + phaseA
            ub = u_bound_a
        else:
            c = w * b2d + phaseB
            ub = u_bound_b
        if w * ub + np.abs(c).max() <= np.pi - MARGIN:
            cc[:, col] = c.astype(np.float32)
            plan.append(("direct", w))
        else:
            cc[:, col] = ((c / TWO_PI + 0.5) * 262144.0).astype(np.float32)
            plan.append(("fold", w))
    return om, al, J, cc, plan


NBLK = HPC * QT + 3  # 8 transposed data tiles + [W1dup|W2dup, cc, vcoef]
BLK_W = HPC * QT
BLK_CC = HPC * QT + 1
BLK_VC = HPC * QT + 2


def build_nc(bV_val, J, plan):
    f32 = mybir.dt.float32
    f16 = mybir.dt.float16
    SIN = mybir.ActivationFunctionType.Sin

    nc = bass.Bass()
    # qk: [128, NBLK, 128] f32. Blocks 0..7: partitions 0:64 = Q^T tile,
    # 64:128 = K^T tile (host pre-transposed). Then [W1dup|W2dup] / cc /
    # vcoef blocks, so the whole constant+input set arrives in ONE DMA
    # (single semaphore -> one wait per operand downstream).
    qk = nc.declare_dram_parameter("qk", [128, NBLK, 128], f32, isOutput=False)
    # out[h, p, qc, k] = scores[h, qc*128+p, k]
    out = nc.declare_dram_parameter("out", [HPC, 128, QT, LK], f32, isOutput=True)

    with tile.TileContext(nc) as tc:
        spsum_cm = tc.tile_pool(name="spsum", bufs=2, space="PSUM")
        spsum = spsum_cm.__enter__()
        ppsum_cm = tc.tile_pool(name="ppsum", bufs=1, space="PSUM")
        ppsum = ppsum_cm.__enter__()
        with (
            tc.tile_pool(name="inp", bufs=1) as inp,
            tc.tile_pool(name="qkt", bufs=1) as qkt_pool,
            tc.tile_pool(name="proj", bufs=1) as proj_pool,
            tc.tile_pool(name="marg", bufs=max(1, sum(1 for p in plan if p[0] != "direct"))) as marg_pool,
            tc.tile_pool(name="mm", bufs=max(1, sum(1 for p in plan if p[0] != "direct"))) as mm_pool,
            tc.tile_pool(name="sout2", bufs=1) as sout2_pool,
            tc.tile_pool(name="atoms", bufs=J) as atom_pool,
            tc.tile_pool(name="braw", bufs=J) as braw_pool,
            tc.tile_pool(name="sout", bufs=1) as sout_pool,
        ):
            insts = {"PE": [], "ACT": [], "DVE": [], "POOL": [], "DMA": []}
            qk_sb = inp.tile([128, NBLK, 128], f32)
            insts["DMA"].append(nc.sync.dma_start(out=qk_sb, in_=qk[:, :, :]))

            # Warm-up touches: one tiny instruction per engine that reads
            # qk_sb, so each engine observes the input-DMA semaphore early
            # and later instructions carry at most ONE new wait (several
            # instruction structs have a single sync-wait slot).
            warm = inp.tile([128, 3], f32, tag="warm")
            insts["POOL"].append(
                nc.gpsimd.tensor_copy(warm[:, 0:1], qk_sb[:, BLK_CC, 0:1]))
            insts["DVE"].append(
                nc.vector.tensor_copy(warm[:, 1:2], qk_sb[:, BLK_CC, 0:1]))
            insts["ACT"].append(
                nc.scalar.copy(warm[:, 2:3], qk_sb[:, BLK_CC, 0:1]))

            # Q^T lives in partitions 0-63 of the data blocks, K^T in
            # 64-127 (host pre-transposed).  Projections contract straight
            # out of the input tile; W2dup sits in partitions 64-127 of the
            # weight block so lhsT/rhs partition bases match.
            aT2 = ppsum.tile([128, HPC, LQ], f32, tag="aT2")
            bT2 = ppsum.tile([128, HPC, LK], f32, tag="bT2")
            for h in range(HPC):
                insts["PE"].append(nc.tensor.matmul(
                    aT2[:, h, :], lhsT=qk_sb[0:64, BLK_W, :],
                    rhs=qk_sb[0:64, h * QT:(h + 1) * QT, :],
                    start=True, stop=True))
                insts["PE"].append(nc.tensor.matmul(
                    bT2[:, h, :], lhsT=qk_sb[64:128, BLK_W, :],
                    rhs=qk_sb[64:128, h * QT:(h + 1) * QT, :],
                    start=True, stop=True))
            aT2_sb = proj_pool.tile([128, HPC * LQ], f32, tag="aT2_sb")
            insts["DVE"].append(nc.vector.tensor_copy(aT2_sb, aT2))
            bT2_sb = proj_pool.tile([128, HPC * LK], f32, tag="bT2_sb")
            insts["DVE"].append(nc.vector.tensor_copy(bT2_sb, bT2))
            ppsum_cm.__exit__(None, None, None)
            spsum2_cm = tc.tile_pool(name="spsum2", bufs=6, space="PSUM")
            spsum2 = spsum2_cm.__enter__()

            negpi = qk_sb[:, BLK_CC, 2 * J:2 * J + 1]
            bvcol = qk_sb[:, BLK_CC, 2 * J + 1:2 * J + 2]
            i32 = mybir.dt.int32
            FSC = 262144.0  # 2^18 phase quantization
            fold_ctr = [0]

            def make_atom(dst, src_sb, col, mod_engine):
                kind = plan[col][0]
                w = plan[col][1]
                cvec = qk_sb[:, BLK_CC, col:col + 1]
                if kind == "direct":
                    insts["ACT"].append(
                        nc.scalar.activation(dst, src_sb, SIN,
                                             bias=cvec, scale=float(w)))
                else:
                    # w32 = i32(u*(2^18*w/2pi) + (c/2pi + .5)*2^18)  [Pool/DVE]
                    # m32 = w32 & 0x3FFFF                            [DVE]
                    # atom = sin((2pi/2^18)*m32 - pi)                [ACT]
                    y = marg_pool.tile([128, HPC * LQ], i32, tag="my")
                    fold_ctr[0] += 1
                    eng, key = ((nc.gpsimd, "POOL") if fold_ctr[0] % 3
                                else (nc.vector, "DVE"))
                    insts[key].append(eng.tensor_scalar(
                        out=y, in0=src_sb,
                        scalar1=float(FSC * w / TWO_PI), scalar2=cvec,
                        op0=mybir.AluOpType.mult, op1=mybir.AluOpType.add))
                    m = mm_pool.tile([128, HPC * LQ], i32, tag="mm")
                    insts["DVE"].append(nc.vector.tensor_scalar(
                        out=m, in0=y, scalar1=0x3FFFF, scalar2=None,
                        op0=mybir.AluOpType.bitwise_and))
                    insts["ACT"].append(
                        nc.scalar.activation(dst, m, SIN,
                                             bias=negpi, scale=float(TWO_PI / FSC)))

            atomsA = []
            atomsB = []
            for j in range(J):
                aA = atom_pool.tile([128, HPC * LQ], f16, tag="atomA")
                make_atom(aA, aT2_sb, j, nc.vector)
                bR = braw_pool.tile([128, HPC * LK], f16, tag="braw")
                make_atom(bR, bT2_sb, J + j, nc.vector)
                aB = atom_pool.tile([128, HPC * LK], f16, tag="atomB")
                insts["DVE"].append(nc.vector.tensor_scalar_mul(
                    aB, bR, qk_sb[:, BLK_VC, j:j + 1]))
                atomsA.append(aA)
                atomsB.append(aB)

            IDENT = mybir.ActivationFunctionType.Identity
            so_h0 = sout_pool.tile([128, QT, LK], f32, tag="so")
            so_h1 = sout2_pool.tile([128, QT, LK], f32, tag="so2")
            sos = [so_h0, so_h1]
            gi = 0
            for h in range(HPC):
                so = sos[h]
                for qc in range(QT):
                    # groups 0-1 and 6-7 use the always-reserved banks;
                    # groups 2-5 take the banks released by tpsum/ppsum
                    # (first reuse happens far from those banks' writers,
                    # so the implied waits collapse to one).
                    if gi < 2:
                        sps = spsum.tile([128, LK], f32, tag="sps")
                    else:
                        sps = spsum2.tile([128, LK], f32, tag="sps2")
                    gi += 1
                    for j in range(J):
                        insts["PE"].append(nc.tensor.matmul(
                            sps,
                            lhsT=atomsA[j][:, h * LQ + qc * 128:
                                            h * LQ + (qc + 1) * 128],
                            rhs=atomsB[j][:, h * LK:(h + 1) * LK],
                            start=(j == 0), stop=(j == J - 1)))
                    if h == 0:
                        insts["DVE"].append(nc.vector.tensor_scalar_add(
                            so[:, qc, :], sps, float(bV_val)))
                    else:
                        insts["ACT"].append(nc.scalar.activation(
                            so[:, qc, :], sps, IDENT, bias=bvcol, scale=1.0))
                insts["DMA"].append(nc.sync.dma_start(out=out[h], in_=so))

            spsum2_cm.__exit__(None, None, None)
            spsum_cm.__exit__(None, None, None)
            # Collector nops: one per producer class, each absorbing one
            # semaphore into the sync engine's observed clock so the
            # framework tail drain needs no multi-sem wait (hardware allows
            # one sync-wait per instruction).
            for key in ("POOL", "ACT", "PE", "DVE"):
                if not insts[key]:
                    continue
                nop = nc.sync.nop(nofuse=True, hint=f"collect_{key}")
                for prod in insts[key]:
                    add_dep_helper(nop.ins, prod.ins, sync=True,
                                   reason=f"tail collector {key}")
            for i, prod in enumerate(insts["DMA"]):
                nop = nc.sync.nop(nofuse=True, hint=f"collect_dma{i}")
                add_dep_helper(nop.ins, prod.ins, sync=True,
                               reason="tail collector dma")
    return nc


def _prep_inputs(Q, K, W1, b1, W2, b2, V, bV):
    B, H, Lq, D_ = Q.shape
    BH = B * H
    Qf = np.ascontiguousarray(Q.reshape(BH, Lq, D_).astype(np.float32))
    Kf = np.ascontiguousarray(K.reshape(BH, Lq, D_).astype(np.float32))

    # data bounds for range-reduction planning (raw projections, bias excluded)
    a_raw = Qf.reshape(-1, D_) @ W1
    b_raw = Kf.reshape(-1, D_) @ W2
    ub_a = float(np.abs(a_raw).max()) + 0.05
    ub_b = float(np.abs(b_raw).max()) + 0.05
    R_need = (ub_a + np.abs(b1).max()) + (ub_b + np.abs(b2).max())

    om, al, J, cc, plan = _plan(b1, b2, ub_a, ub_b, R_need)

    consts = np.zeros((128, 3, 128), np.float32)
    consts[0:64, 0, :] = np.concatenate([W1, W1], axis=1)
    consts[64:128, 0, :] = np.concatenate([W2, W2], axis=1)
    consts[:, 1, 0:2 * J] = cc
    consts[:, 1, 2 * J] = -np.pi
    consts[:, 1, 2 * J + 1] = np.float32(bV[0])
    Vd = np.concatenate([V[:, 0], V[:, 0]])
    consts[:, 2, 0:J] = al[None, :] * Vd[:, None]

    in_maps = []
    for c in range(N_CORES):
        qk = np.empty((128, NBLK, 128), np.float32)
        for i in range(HPC):
            h = HPC * c + i
            qt = Qf[h].T
            kt = Kf[h].T
            for t in range(QT):
                qk[0:64, i * QT + t, :] = qt[:, t * 128:(t + 1) * 128]
                qk[64:128, i * QT + t, :] = kt[:, t * 128:(t + 1) * 128]
        qk[:, HPC * QT:, :] = consts
        in_maps.append({"qk": qk})
    return in_maps, J, plan


def _run(inputs, trace=False, **kwargs):
    Q = np.asarray(inputs["Q"], np.float32)
    K = np.asarray(inputs["K"], np.float32)
    W1 = np.asarray(inputs["W1"], np.float32)
    b1 = np.asarray(inputs["b1"], np.float32)
    W2 = np.asarray(inputs["W2"], np.float32)
    b2 = np.asarray(inputs["b2"], np.float32)
    V = np.asarray(inputs["V"], np.float32)
    bV = np.asarray(inputs["bV"], np.float32)

    in_maps, J, plan = _prep_inputs(Q, K, W1, b1, W2, b2, V, bV)
    nc = build_nc(float(bV[0]), J, plan)
    res = run_bass_kernel_spmd(nc, in_maps, list(range(N_CORES)),
                               trace=trace, **kwargs)

    B, H, Lq, _ = Q.shape
    out = np.empty((B * H, Lq, LK), np.float32)
    for c in range(N_CORES):
        o = res.results[c]["out"]          # [HPC, 128, QT, LK]
        out[HPC * c:HPC * (c + 1)] = (
            o.transpose(0, 2, 1, 3).reshape(HPC, Lq, LK))
    return out.reshape(B, H, Lq, LK), res


def kernel(**inputs) -> np.ndarray:
    out, _ = _run(inputs, trace=False)
    return out
